# revision 3
# baseline (speedup 1.0000x reference)
"""Trainium2 Bass kernel for ClusteringMMD — v2.

Differences vs v1 baseline:
  - host casts the 0/1 adjacency to fp8e4 (exact) -> 4x less HBM read,
    and the on-device ScalarE f32->fp8 cast disappears entirely
  - only the upper block-triangle of A^2 is computed (A, A^2, and
    A^2*A are all symmetric): 10 of 16 [128,128] blocks = 62.5% of
    the matmul stream
  - a constant ones column rides at col 512 of the moving operand, so
    deg (= column-sum of A) accumulates in the last PSUM column of
    each row-block for free
  - tri2 row-parts come from the DVE scalar_tensor_tensor accumulator
    (x = A^2 * A in bf16 + rowsum); the missing lower-block parts are
    column-sums of the symmetric Hadamard tiles, computed as tiny
    ones-vector matmuls (FWL) on the PE into a PSUM accumulator
  - host adds row-part + col-part (exact f32 integers), then does the
    tiny histogram + MMD exactly as before

Layout: a[g, p, t*528 + n] = A_g[t*128 + p, n] for n<512; col 512 = 1.0
(the ones column); cols 513..527 = 0 padding (DoubleRow requires the
k-pair step to be a multiple of 16 bytes).
"""

import json
import numpy as np
import ml_dtypes

B = 128
N = 512
BINS = 100
SIGMA = 1.0
N_CORES = 8
PER = B // N_CORES          # graphs per input tensor per core
GP = 2 * PER                # graphs per core (adj_1 shard + adj_2 shard)
P = 128
T = N // P                  # 4 row-blocks
TP = 528                    # padded row-block pitch (16-aligned, >= 513)

WAIT_CAP = 1                # max sync waits this walrus accepts per inst

_NC_CACHE = {}

# upper-triangle moving slices: (m, start_col, width_incl_ones, has_ones)
# m=0 is split at 256 so the ones column lands in a <=512-col PSUM bank
# and so mini-matmul 128-col sub-blocks never straddle the split.
MSLICES = [
    (0, 0, 256, False),     # x0A: A^2[0-block rows, cols 0..255]
    (0, 256, 257, True),    # x0B: cols 256..511 + deg(m=0)
    (1, 128, 385, True),    # x1:  cols 128..511 + deg(m=1)
    (2, 256, 257, True),    # x2:  cols 256..511 + deg(m=2)
    (3, 384, 129, True),    # x3:  cols 384..511 + deg(m=3)
]
# colsum minis: for col-block nb, list of (mslice index, local offset)
MINIS = {
    1: [(0, 128)],
    2: [(1, 0), (2, 128)],
    3: [(1, 128), (2, 256), (3, 128)],
}
N_SL = len(MSLICES)         # 5 STT accum cols per graph
N_DEG = 4
N_LO = 3


def _split_waits(bir_json, cap=WAIT_CAP):
    """Rewrite BIR JSON so no instruction carries more than `cap` sync
    waits; excess waits move to NoOps inserted just before it on the same
    engine (per-engine program order is list order within a block)."""
    m = json.loads(bir_json)
    ctr = 0
    for fn in m.get("functions", []):
        for blk in fn.get("blocks", []):
            out = []
            changed = False
            for ins in blk.get("instructions", []):
                si = ins.get("sync_info")
                waits = (si or {}).get("on_wait") or []
                if len(waits) > cap:
                    changed = True
                    for i in range(0, len(waits) - cap, cap):
                        ctr += 1
                        out.append(
                            {
                                "debug": ins.get("debug", 0),
                                "engine": ins["engine"],
                                "ins": [],
                                "name": f"WSPLIT-{ctr}",
                                "opcode": "NoOp",
                                "outs": [],
                                "text_hint": "wait_split",
                                "sync_info": {
                                    "on_wait": waits[i : i + cap],
                                    "on_update": [],
                                },
                            }
                        )
                    si["on_wait"] = waits[len(waits) - cap :]
                out.append(ins)
            if changed:
                blk["instructions"] = out
    return json.dumps(m).encode()


def _patch_compiler_wait_split():
    import concourse.bass_utils as bu
    import concourse.bass2jax as b2j

    if getattr(bu, "_wait_split_patched", False):
        return
    orig = bu.compile_bir_kernel

    def wrapped(bir_json, tmpdir, neff_name="file.neff"):
        return orig(_split_waits(bir_json), tmpdir, neff_name)

    bu.compile_bir_kernel = wrapped
    b2j.compile_bir_kernel = wrapped
    bu._wait_split_patched = True


def build_nc(gp=GP):
    import concourse.bass as bass
    import concourse.mybir as mybir
    from concourse.tile import TileContext
    from contextlib import ExitStack

    _patch_compiler_wait_split()
    dt = mybir.dt

    nc = bass.Bass(
        "TRN2", target_bir_lowering=False, debug=False, num_devices=N_CORES
    )
    a = nc.declare_dram_parameter("a", [gp, P, T * TP], dt.float8e4, isOutput=False)
    # tri2 row-parts: 5 accum cols per graph (partition-major)
    ot = nc.declare_dram_parameter("ot", [P, gp * N_SL], dt.float32, isOutput=True)
    # tri2 lower col-parts: 3 cols per graph
    ol = nc.declare_dram_parameter("ol", [P, gp * N_LO], dt.float32, isOutput=True)
    # deg: 4 cols per graph
    od = nc.declare_dram_parameter("od", [P, gp * N_DEG], dt.float32, isOutput=True)

    with TileContext(nc) as tc, ExitStack() as ctx:
        pconst = ctx.enter_context(tc.tile_pool(name="const", bufs=1))
        paf = ctx.enter_context(tc.tile_pool(name="af", bufs=8))
        pxs = ctx.enter_context(tc.tile_pool(name="xs", bufs=6 * N_SL))
        pps = ctx.enter_context(tc.tile_pool(name="ps", bufs=6, space="PSUM"))
        ppt = ctx.enter_context(tc.tile_pool(name="pt", bufs=2, space="PSUM"))

        ones_bf = pconst.tile([P, 1], dt.bfloat16)
        nc.vector.memset(ones_bf[:], 1.0)
        st_all = pconst.tile([P, gp * N_SL], dt.float32)
        lo_all = pconst.tile([P, gp * N_LO], dt.float32)
        dg_all = pconst.tile([P, gp * N_DEG], dt.float32)

        # per-graph deferred work, keyed by graph index
        pend = {}

        def emit_minis(g):
            """colsum mini-matmuls + staging copies for graph g."""
            xs, ps_tiles = pend.pop(g)
            pt = ppt.tile([P, N_LO], dt.float32)
            for nb, blocks in MINIS.items():
                nbk = len(blocks)
                for bi, (si, off) in enumerate(blocks):
                    nc.tensor.matmul(
                        pt[:, nb - 1 : nb],
                        xs[si][:, off : off + P],
                        ones_bf[:],
                        start=(bi == 0),
                        stop=(bi == nbk - 1),
                    )
            nc.scalar.copy(lo_all[:, g * N_LO : (g + 1) * N_LO], pt[:])

        for g in range(gp):
            af = paf.tile([P, T, TP], dt.float8e4)
            nc.sync.dma_start(
                out=af[:],
                in_=a[g].rearrange("p (t n) -> p t n", t=T),
            )
            ps_tiles = []
            xs = []
            # upper-triangle DoubleRow matmuls (+ ones column for deg)
            for si, (m, c0, w, has_ones) in enumerate(MSLICES):
                ps = pps.tile([P, 512], dt.float32)
                for kk in range(T // 2):
                    nc.tensor.matmul(
                        ps[:, 0:w],
                        af[:, 2 * kk : 2 * kk + 2, m * P : (m + 1) * P],
                        af[:, 2 * kk : 2 * kk + 2, c0 : c0 + w],
                        start=(kk == 0),
                        stop=(kk == T // 2 - 1),
                        perf_mode=mybir.MatmulPerfMode.DoubleRow,
                    )
                ps_tiles.append(ps)
            # minis for graph g-2 (x tiles ready; keeps PE from stalling
            # on the concurrent DVE STTs)
            if g - 2 in pend:
                emit_minis(g - 2)
            # Hadamard + rowsum on DVE
            for si, (m, c0, w, has_ones) in enumerate(MSLICES):
                sw = w - 1 if has_ones else w
                x = pxs.tile([P, sw], dt.bfloat16)
                nc.vector.scalar_tensor_tensor(
                    x[:],
                    ps_tiles[si][:, 0:sw],
                    1.0,
                    af[:, m, c0 : c0 + sw],
                    op0=mybir.AluOpType.mult,
                    op1=mybir.AluOpType.mult,
                    accum_out=st_all[:, g * N_SL + si : g * N_SL + si + 1],
                )
                xs.append(x)
            # deg staging (ScalarE; PSUM -> SBUF) as soon as blocks finish
            di = 0
            for si, (m, c0, w, has_ones) in enumerate(MSLICES):
                if has_ones:
                    nc.scalar.copy(
                        dg_all[:, g * N_DEG + di : g * N_DEG + di + 1],
                        ps_tiles[si][:, w - 1 : w],
                    )
                    di += 1
            pend[g] = (xs, ps_tiles)
            # batched output DMA (SWDGE so input queues aren't blocked)
            if (g + 1) % 16 == 0:
                g0 = g - 15
                if g - 1 in pend:
                    emit_minis(g - 1)
                if g in pend:
                    emit_minis(g)
                nc.gpsimd.dma_start(
                    out=ot[:, g0 * N_SL : (g + 1) * N_SL],
                    in_=st_all[:, g0 * N_SL : (g + 1) * N_SL],
                )
                nc.gpsimd.dma_start(
                    out=ol[:, g0 * N_LO : (g + 1) * N_LO],
                    in_=lo_all[:, g0 * N_LO : (g + 1) * N_LO],
                )
                nc.gpsimd.dma_start(
                    out=od[:, g0 * N_DEG : (g + 1) * N_DEG],
                    in_=dg_all[:, g0 * N_DEG : (g + 1) * N_DEG],
                )
    return nc


def _get_nc():
    key = GP
    if key not in _NC_CACHE:
        _NC_CACHE[key] = build_nc(key)
    return _NC_CACHE[key]


def _permute_shard(shard):
    """[gp, 512, 512] f32 -> [gp, 128, T*528] fp8 with the layout
    a[g, p, t*528+n] = A[g, t*128+p, n], ones at n=512, zeros beyond."""
    gp = shard.shape[0]
    out = np.zeros((gp, P, T, TP), dtype=ml_dtypes.float8_e4m3fn)
    perm = shard.reshape(gp, T, P, N).transpose(0, 2, 1, 3)
    out[:, :, :, :N] = perm.astype(ml_dtypes.float8_e4m3fn)
    out[:, :, :, N] = np.asarray(1.0, dtype=ml_dtypes.float8_e4m3fn)
    return out.reshape(gp, P, T * TP)


def run_device(adj_1, adj_2, trace=False):
    """Run the bass kernel on 8 cores; returns (tri2, deg) for each input
    tensor as [B, N] f32 arrays, plus the BassKernelResults."""
    from concourse.bass_utils import run_bass_kernel_spmd

    nc = _get_nc()
    in_maps = []
    for c in range(N_CORES):
        shard = np.concatenate(
            [adj_1[c * PER : (c + 1) * PER], adj_2[c * PER : (c + 1) * PER]],
            axis=0,
        )
        in_maps.append({"a": _permute_shard(shard)})
    res = run_bass_kernel_spmd(nc, in_maps, list(range(N_CORES)), trace=trace)

    tri = np.empty((N_CORES, GP, N), np.float32)
    deg = np.empty((N_CORES, GP, N), np.float32)
    for ci, r in enumerate(res.results):
        st = r["ot"].reshape(P, GP, N_SL)     # [p, g, slice]
        lo = r["ol"].reshape(P, GP, N_LO)     # [p, g, nb-1]
        dg = r["od"].reshape(P, GP, N_DEG)    # [p, g, m]
        for g in range(GP):
            # row parts: m=0 split in slices 0+1, m=1..3 in slices 2..4
            tri[ci, g, 0:P] = st[:, g, 0] + st[:, g, 1]
            for m in range(1, T):
                tri[ci, g, m * P : (m + 1) * P] = (
                    st[:, g, m + 1] + lo[:, g, m - 1]
                )
            for m in range(T):
                deg[ci, g, m * P : (m + 1) * P] = dg[:, g, m]
    tri2_1 = tri[:, :PER].reshape(B, N)
    tri2_2 = tri[:, PER:].reshape(B, N)
    deg_1 = deg[:, :PER].reshape(B, N)
    deg_2 = deg[:, PER:].reshape(B, N)
    return (tri2_1, deg_1), (tri2_2, deg_2), res


def _hist(tri2, deg):
    # bit-exact f32 replication of the reference binning
    tri2 = tri2.astype(np.float32)
    deg = deg.astype(np.float32)
    denom = deg * (deg - np.float32(1.0))
    c = np.where(
        denom > 0,
        tri2 / np.maximum(denom, np.float32(1.0)),
        np.float32(0.0),
    ).astype(np.float32)
    idx = np.clip((c * np.float32(BINS)).astype(np.int32), 0, BINS - 1)
    hist = np.zeros((idx.shape[0], BINS), np.float32)
    np.add.at(hist, (np.arange(idx.shape[0])[:, None], idx), np.float32(1.0))
    return hist


def _mmd(x, y):
    x = x.astype(np.float64)
    y = y.astype(np.float64)

    def kmat(a, b):
        sq = (
            (a * a).sum(-1)[:, None]
            + (b * b).sum(-1)[None, :]
            - 2.0 * (a @ b.T)
        )
        return np.exp(-np.maximum(sq, 0.0) / (2.0 * SIGMA * SIGMA))

    return kmat(x, x).mean() + kmat(y, y).mean() - 2.0 * kmat(x, y).mean()


def kernel(adj_1, adj_2):
    (t1, d1), (t2, d2), _ = run_device(adj_1, adj_2)
    h1 = _hist(t1, d1)
    h2 = _hist(t2, d2)
    return np.float32(_mmd(h1, h2))
